# revision 48
# baseline (speedup 1.0000x reference)
"""MoE layer (8 experts, top-2) on 8 TRN2 NeuronCores.

Strategy: data-parallel over tokens with host-side routing-aware sharding
("all-to-all tokens by top-k assignment" done on the host). The host
routes tokens (fp32, bit-matching the reference selection), assigns
tokens to cores greedily so per-(core, expert) counts are nearly equal,
derives per-expert slot capacities CAPS[e] from the realized maxima, and
ships each core its tokens PRE-SORTED into expert-major slot order
(d-major layout, bf16), so the device runs no gather/compaction at all.

On device: the two big GEMMs per expert start immediately (slot 0 data +
first weight quarter land ~10us in); the router reruns per-slot in bf16
(logitsT = rw.T @ xg, stationary rw) to compute the renormalized top-2
gate p = sigmoid(2*l_own - m1 - m2) — a smooth function of logits, so
bf16 is safe (the discrete top-2 selection lives on the host) — and the
gate is fused into the GEMM2 bias epilogue ((psum + b2) * gate on DVE).
Gated y-slots stream out; the host adds rows into the final output
(pure unshard work: the slot->token map is the host's own sharding).

Self-contained: hardcodes shapes B=4, S=2048, D=1024, F=4096, E=8, K=2.
"""
import sys

for p in ("/opt/trn_rl_repo",):
    if p not in sys.path:
        sys.path.insert(0, p)

import numpy as np
import ml_dtypes

import concourse.mybir as mybir
from concourse import bacc
from concourse.bass_utils import run_bass_kernel_spmd
from concourse.tile import TileContext
from concourse.tile_rust import add_dep_helper

B, S, D, F, E = 4, 2048, 1024, 4096, 8
N = B * S            # 8192 tokens total
NC = 8               # cores
NT = N // NC         # 1024 tokens per core
KD = D // 128        # 8 contraction tiles over D
MF = F // 128        # 32 f tiles
NQ = 4               # weight streaming quarters per expert per GEMM
SUBF = F // NQ       # 1024 f columns per w1 quarter
SUBD = D // NQ       # 256 d columns per w2 quarter
F32 = mybir.dt.float32
BF16 = mybir.dt.bfloat16
NPBF16 = ml_dtypes.bfloat16

_GELU = mybir.ActivationFunctionType.Gelu


def build_nc(caps):
    """caps: per-expert slot capacities (each %16)."""
    caps = list(caps)
    assert len(caps) == E and all(c % 16 == 0 for c in caps)
    CAPX = max(caps)               # uniform per-expert stride
    NS = E * CAPX                  # unified slot space
    NCH = NS // 128                # 128-slot chunks (NS % 128 == 0)
    assert NS % 128 == 0

    nc = bacc.Bacc()
    xg_dr = nc.declare_dram_parameter("xg", [E, 128, KD * CAPX], BF16,
                                      isOutput=False)
    rw_dr = nc.declare_dram_parameter("rw", [128, KD * E], BF16, isOutput=False)
    rb_dr = nc.declare_dram_parameter("rb", [E, 1], F32, isOutput=False)
    w1_dr = nc.declare_dram_parameter("w1b", [E, NQ, 128, KD * SUBF], BF16,
                                      isOutput=False)
    # expert 0's first quarter in m-tile-major layout: each 256KB m-tile is
    # one contiguous DMA, so the very first GEMM matmuls start ~10us in
    w1m_dr = nc.declare_dram_parameter("w1e0m", [SUBF // 128, 128, KD * 128],
                                       BF16, isOutput=False)
    b1_dr = nc.declare_dram_parameter("b1r", [E, 128, MF], F32, isOutput=False)
    w2_dr = nc.declare_dram_parameter("w2b", [E, NQ, 128, MF * SUBD], BF16,
                                      isOutput=False)
    b2_dr = nc.declare_dram_parameter("b2r", [E, 128, KD], F32, isOutput=False)
    id_dr = nc.declare_dram_parameter("ident", [128, 128], F32, isOutput=False)
    om_dr = nc.declare_dram_parameter("ownmask", [128, NCH * E], F32,
                                      isOutput=False)
    out_dr = nc.declare_dram_parameter("out", [E, 128, KD * CAPX], F32,
                                       isOutput=True)

    with TileContext(nc) as tc:
        with tc.tile_pool(name="persist", bufs=1) as pp:
            # consts on the scalar queue: the sync queue leads with weights
            rw_sb = pp.tile([128, KD, E], BF16)
            nc.scalar.dma_start(out=rw_sb[:].rearrange("p k e -> p (k e)"),
                                in_=rw_dr[:])
            rb_sb = pp.tile([E, 1], F32)
            nc.scalar.dma_start(out=rb_sb[:], in_=rb_dr[:])
            ident = pp.tile([128, 128], F32)
            nc.scalar.dma_start(out=ident[:], in_=id_dr[:])
            ownmask = pp.tile([128, NCH, E], F32)
            nc.scalar.dma_start(out=ownmask[:].rearrange("p c e -> p (c e)"),
                                in_=om_dr[:])
            ones_row = pp.tile([1, 128], F32)
            nc.vector.memset(ones_row[:], 1.0)

            # pre-sorted tokens, d-major: xg[p, e, k, s] = x[tok(e,s), 128k+p]
            # on the gpsimd queue; blocks 0-3 up front, blocks 4-7 paced off
            # early expert-0 activations (they land ~16-23us, well before the
            # router consumes them at ~44us) so the congested first ~25us of
            # HBM bandwidth feeds the expert-0 weight stream
            xg = pp.tile([128, E, KD, CAPX], BF16)
            for e in range(4):
                nc.gpsimd.dma_start(out=xg[:, e, :, :].rearrange("p k s -> p (k s)"),
                                    in_=xg_dr[e])

            probs_all = pp.tile([128, NS], F32)
            pflat = pp.tile([1, NS], F32)

            with (
                tc.tile_pool(name="rt_sb", bufs=2) as rsp,
                tc.tile_pool(name="rt1_sb", bufs=1) as rp1,
                tc.tile_pool(name="ps_rt", bufs=2, space="PSUM") as prt,
                tc.tile_pool(name="w1p", bufs=3) as wp1,
                tc.tile_pool(name="w1pm", bufs=3) as wp1m,
                tc.tile_pool(name="w2p", bufs=3) as wp2,
                tc.tile_pool(name="ht", bufs=1) as hp,
                tc.tile_pool(name="yt", bufs=2) as yp,
                tc.tile_pool(name="bias", bufs=2) as bp,
                tc.tile_pool(name="ps_g1", bufs=3, space="PSUM") as ps1,
                tc.tile_pool(name="ps_g2", bufs=3, space="PSUM") as ps2,
            ):
                lgs = rp1.tile([8, NS], F32)       # per-slot logitsT
                lg_all = rp1.tile([128, NCH, E], F32)

                def emit_router():
                    # logitsT = rw.T @ xg per expert block, bf16, rb added
                    for e in range(E):
                        psr = prt.tile([128, 512], F32, tag="rt", name=f"rt{e}")
                        for k in range(KD):
                            nc.tensor.matmul(psr[:8, :caps[e]], rw_sb[:, k, :],
                                             xg[:, e, k, :caps[e]],
                                             start=(k == 0), stop=(k == KD - 1))
                        nc.vector.tensor_scalar(
                            out=lgs[:, e * CAPX:e * CAPX + caps[e]],
                            in0=psr[:8, :caps[e]], scalar1=rb_sb[:, 0:1],
                            scalar2=None, op0=mybir.AluOpType.add)
                    for c in range(NCH):
                        tps = prt.tile([128, 512], F32, tag="rt", name=f"tp{c}")
                        nc.tensor.transpose(tps[:, :8], lgs[:, c * 128:(c + 1) * 128],
                                            ident[:8, :8])
                        nc.vector.tensor_copy(lg_all[:, c, :], tps[:, :8])

                def emit_probs():
                    # p = sigmoid(2*l_own - m1 - m2) per slot (smooth in logits)
                    m1 = rp1.tile([128, NCH], F32)
                    nc.vector.tensor_reduce(m1[:], lg_all[:],
                                            axis=mybir.AxisListType.X,
                                            op=mybir.AluOpType.max)
                    is1 = rp1.tile([128, NCH, E], F32)
                    nc.vector.tensor_tensor(
                        out=is1[:], in0=lg_all[:],
                        in1=m1[:].unsqueeze(2).broadcast_to([128, NCH, E]),
                        op=mybir.AluOpType.is_equal)
                    l2 = rp1.tile([128, NCH, E], F32)
                    nc.vector.scalar_tensor_tensor(out=l2[:], in0=is1[:],
                                                   scalar=-1e30, in1=lg_all[:],
                                                   op0=mybir.AluOpType.mult,
                                                   op1=mybir.AluOpType.add)
                    m2 = rp1.tile([128, NCH], F32)
                    nc.vector.tensor_reduce(m2[:], l2[:],
                                            axis=mybir.AxisListType.X,
                                            op=mybir.AluOpType.max)
                    lo = rp1.tile([128, NCH, E], F32)
                    nc.vector.tensor_tensor(out=lo[:], in0=lg_all[:], in1=ownmask[:],
                                            op=mybir.AluOpType.mult)
                    low = rp1.tile([128, NCH], F32)
                    nc.vector.tensor_reduce(low[:], lo[:],
                                            axis=mybir.AxisListType.X,
                                            op=mybir.AluOpType.add)
                    arg = rp1.tile([128, NCH], F32)
                    nc.vector.tensor_tensor(out=arg[:], in0=m1[:], in1=m2[:],
                                            op=mybir.AluOpType.add)
                    nc.vector.scalar_tensor_tensor(out=arg[:], in0=low[:],
                                                   scalar=2.0, in1=arg[:],
                                                   op0=mybir.AluOpType.mult,
                                                   op1=mybir.AluOpType.subtract)
                    pch = rp1.tile([128, NCH], F32)
                    nc.scalar.activation(pch[:], arg[:],
                                         mybir.ActivationFunctionType.Sigmoid,
                                         bias=0.0, scale=1.0)
                    # [128, NCH] -> [NCH, 128] -> flat [1, NS] -> bcast [128, NS]
                    tpp = prt.tile([128, 512], F32, tag="rt", name="tq")
                    nc.tensor.transpose(tpp[:NCH, :128], pch[:], ident[:])
                    pT = rp1.tile([NCH, 128], F32)
                    nc.vector.tensor_copy(pT[:], tpp[:NCH, :128])
                    for c in range(NCH):
                        nc.scalar.dma_start(out=pflat[0:1, c * 128:(c + 1) * 128],
                                            in_=pT[c:c + 1, :])
                    for e in range(E):
                        pb = prt.tile([128, 512], F32, tag="rt", name=f"pb{e}")
                        assert caps[e] <= 512
                        nc.tensor.matmul(pb[:, :caps[e]], ones_row[:],
                                         pflat[0:1, e * CAPX:e * CAPX + caps[e]],
                                         start=True, stop=True)
                        nc.vector.tensor_copy(
                            probs_all[:, e * CAPX:e * CAPX + caps[e]],
                            pb[:, :caps[e]])

                for e in range(E):
                    cap = caps[e]
                    b1_sb = bp.tile([128, MF], F32, tag="b1")
                    b2_sb = bp.tile([128, KD], F32, tag="b2")
                    if e == 0:
                        # first weight m-tile outranks the biases on the queue
                        w1m0 = wp1m.tile([128, KD, 128], BF16, tag="w1m")
                        nc.sync.dma_start(out=w1m0[:].rearrange("p k f -> p (k f)"),
                                          in_=w1m_dr[0])
                    nc.sync.dma_start(out=b1_sb[:], in_=b1_dr[e])
                    nc.sync.dma_start(out=b2_sb[:], in_=b2_dr[e])

                    # GEMM1 + bias + gelu -> hT [128, MF, cap] bf16
                    hT = hp.tile([128, MF, CAPX], BF16, tag="hT")
                    for q in range(NQ):
                        if not (e == 0 and q == 0):
                            w1q = wp1.tile([128, KD, SUBF], BF16, tag="w1q")
                            nc.sync.dma_start(out=w1q[:].rearrange("p k f -> p (k f)"),
                                              in_=w1_dr[e, q])
                        for mi in range(SUBF // 128):
                            m = q * (SUBF // 128) + mi
                            if e == 0 and q == 0:
                                if mi == 0:
                                    w1m = w1m0
                                else:
                                    w1m = wp1m.tile([128, KD, 128], BF16, tag="w1m")
                                    nc.sync.dma_start(
                                        out=w1m[:].rearrange("p k f -> p (k f)"),
                                        in_=w1m_dr[mi])
                                wsl = w1m
                                wof = 0
                            else:
                                wsl = w1q
                                wof = mi * 128
                            ps = ps1.tile([128, CAPX], F32, tag="g1",
                                          name=f"g1_{e}_{m}")
                            for k in range(KD):
                                nc.tensor.matmul(
                                    ps[:, :cap],
                                    wsl[:, k, wof:wof + 128],
                                    xg[:, e, k, :cap],
                                    start=(k == 0), stop=(k == KD - 1))
                            act = nc.scalar.activation(hT[:, m, :cap], ps[:, :cap],
                                                       _GELU, bias=b1_sb[:, m:m + 1],
                                                       scale=1.0)
                            if e == 0 and q == 0 and m in (1, 3, 5, 7):
                                ge = 4 + (m - 1) // 2
                                xd = nc.gpsimd.dma_start(
                                    out=xg[:, ge, :, :].rearrange("p k s -> p (k s)"),
                                    in_=xg_dr[ge])
                                add_dep_helper(xd.ins, act.ins,
                                               reason="pace xg tail vs weights")
                        if e == 0 and q == 2:
                            emit_router()
                        if e == 0 and q == 3:
                            emit_probs()

                    # GEMM2 with fused (psum + b2) * gate epilogue
                    yT = yp.tile([128, KD, CAPX], F32, tag="yT")
                    for dq in range(NQ):
                        w2q = wp2.tile([128, MF, SUBD], BF16, tag="w2q")
                        nc.sync.dma_start(out=w2q[:].rearrange("p k d -> p (k d)"),
                                          in_=w2_dr[e, dq])
                        for mi in range(SUBD // 128):
                            m = dq * (SUBD // 128) + mi
                            ps = ps2.tile([128, CAPX], F32, tag="g2",
                                          name=f"g2_{e}_{m}")
                            for k2 in range(MF):
                                nc.tensor.matmul(
                                    ps[:, :cap],
                                    w2q[:, k2, mi * 128:(mi + 1) * 128],
                                    hT[:, k2, :cap],
                                    start=(k2 == 0), stop=(k2 == MF - 1))
                            nc.vector.scalar_tensor_tensor(
                                out=yT[:, m, :cap], in0=ps[:, :cap],
                                scalar=b2_sb[:, m:m + 1],
                                in1=probs_all[:, e * CAPX:e * CAPX + cap],
                                op0=mybir.AluOpType.add,
                                op1=mybir.AluOpType.mult)
                        m0 = dq * (SUBD // 128)
                        if e == E - 1:
                            # finer final-expert writes shorten the tail drain
                            for m in range(m0, m0 + SUBD // 128):
                                nc.gpsimd.dma_start(
                                    out=out_dr[e][:, m * CAPX:(m + 1) * CAPX],
                                    in_=yT[:, m, :])
                        else:
                            nc.gpsimd.dma_start(
                                out=out_dr[e][:, m0 * CAPX:(m0 + SUBD // 128) * CAPX],
                                in_=yT[:, m0:m0 + SUBD // 128, :]
                                .rearrange("p a b -> p (a b)"))

    nc.finalize()
    return nc


def route_and_balance(x_flat, router_w, router_b):
    """Host fp32 routing (matches the reference selection) + greedy packing."""
    rwf = np.asarray(router_w, dtype=np.float32)
    logits = x_flat @ rwf + np.asarray(router_b, dtype=np.float32)
    top2 = np.argsort(-logits, axis=1)[:, :2]
    cnt = np.zeros((NC, E), np.int64)
    room = np.full(NC, NT, np.int64)
    perm = [[] for _ in range(NC)]
    for t in range(N):
        a, b = top2[t]
        best, bkey = None, None
        for c in range(NC):
            if room[c] == 0:
                continue
            key = (max(cnt[c, a], cnt[c, b]), cnt[c, a] + cnt[c, b], NT - room[c])
            if bkey is None or key < bkey:
                bkey, best = key, c
        perm[best].append(t)
        cnt[best, a] += 1
        cnt[best, b] += 1
        room[best] -= 1
    perm = np.array(perm, dtype=np.int64)
    caps = tuple(int(-(-(int(cnt[:, e].max())) // 16) * 16) for e in range(E))
    return perm, top2, caps


def make_in_maps(x, router_w, router_b, w1, b1, w2, b2):
    x_flat = np.ascontiguousarray(np.asarray(x, dtype=np.float32).reshape(N, D))
    perm, top2, caps = route_and_balance(x_flat, router_w, router_b)
    CAPX = max(caps)
    NS = E * CAPX
    NCH = NS // 128
    ident = np.eye(128, dtype=np.float32)
    b1r = np.ascontiguousarray(
        np.asarray(b1, dtype=np.float32).reshape(E, MF, 128).transpose(0, 2, 1))
    b2r = np.ascontiguousarray(
        np.asarray(b2, dtype=np.float32).reshape(E, KD, 128).transpose(0, 2, 1))
    w1b = np.ascontiguousarray(
        np.asarray(w1, dtype=np.float32).reshape(E, KD, 128, NQ, SUBF)
        .transpose(0, 3, 2, 1, 4).reshape(E, NQ, 128, KD * SUBF)
        .astype(NPBF16))
    w2b = np.ascontiguousarray(
        np.asarray(w2, dtype=np.float32).reshape(E, MF, 128, NQ, SUBD)
        .transpose(0, 3, 2, 1, 4).reshape(E, NQ, 128, MF * SUBD)
        .astype(NPBF16))
    # w1e0m[mi, p, (k, c)] = w1[0, 128k + p, 128mi + c]  (m-tile-major q0)
    w1e0m = np.ascontiguousarray(
        np.asarray(w1[0], dtype=np.float32).reshape(KD, 128, MF, 128)
        .transpose(2, 1, 0, 3)[:SUBF // 128].reshape(SUBF // 128, 128, KD * 128)
        .astype(NPBF16))
    rw_re = np.ascontiguousarray(
        np.asarray(router_w, dtype=np.float32).reshape(KD, 128, E)
        .transpose(1, 0, 2).reshape(128, KD * E)).astype(NPBF16)
    rb_re = np.ascontiguousarray(
        np.asarray(router_b, dtype=np.float32).reshape(E, 1))
    common = dict(rw=rw_re, rb=rb_re, w1b=w1b, w1e0m=w1e0m, b1r=b1r, w2b=w2b,
                  b2r=b2r, ident=ident)
    in_maps = []
    slot_tok = []      # per core: local token id per slot (-1 = pad)
    for c in range(NC):
        m = dict(common)
        xs = x_flat[perm[c]].astype(NPBF16)
        t2c = top2[perm[c]]    # [NT, 2] expert pairs of this core's tokens
        st = np.full((E, CAPX), -1, np.int64)
        fill = np.zeros(E, np.int64)
        for lid in range(NT):
            for e in t2c[lid]:
                st[e, fill[e]] = lid
                fill[e] += 1
        slot_tok.append(st)
        stc = np.where(st < 0, 0, st)    # pad slots read token 0 (ignored)
        xsl = xs[stc.reshape(-1)]        # [NS, D] bf16
        # xg[e, p, (k, s)] = xsl[e*CAPX + s, 128k + p]
        m["xg"] = np.ascontiguousarray(
            xsl.reshape(E, CAPX, KD, 128).transpose(0, 3, 2, 1)
            .reshape(E, 128, KD * CAPX))
        # ownmask[p, c, e] = 1 iff slot c*128+p belongs to expert e (not pad)
        om = np.zeros((128, NCH, E), np.float32)
        sl = np.arange(NS)
        eo = sl // CAPX
        valid = (st.reshape(-1) >= 0)
        om[sl % 128, sl // 128, eo] = valid.astype(np.float32)
        m["ownmask"] = np.ascontiguousarray(om.reshape(128, NCH * E))
        in_maps.append(m)
    return in_maps, perm, caps, slot_tok


def assemble(results, perm, caps, slot_tok):
    CAPX = max(caps)
    out = np.zeros((N, D), np.float32)
    for c in range(NC):
        y = results[c]["out"].reshape(E, 128, KD, CAPX)
        for e in range(E):
            k_e = int((slot_tok[c][e] >= 0).sum())
            rows = np.ascontiguousarray(
                y[e, :, :, :k_e].transpose(2, 1, 0).reshape(k_e, D))
            np.add.at(out, perm[c][slot_tok[c][e][:k_e]], rows)
    return out


_nc_cache = {}


def get_nc(caps):
    if caps not in _nc_cache:
        _nc_cache[caps] = build_nc(caps)
    return _nc_cache[caps]


def kernel(x, router_w, router_b, w1, b1, w2, b2, **extra):
    in_maps, perm, caps, slot_tok = make_in_maps(x, router_w, router_b,
                                                 w1, b1, w2, b2)
    nc = get_nc(caps)
    res = run_bass_kernel_spmd(nc, in_maps, list(range(NC)))
    out = assemble(res.results, perm, caps, slot_tok)
    return out.reshape(B, S, D)


# revision 51
# speedup vs baseline: 1.2293x; 1.2293x over previous
"""MoE layer (8 experts, top-2) on 8 TRN2 NeuronCores.

Strategy: data-parallel over tokens with host-side routing-aware sharding
("all-to-all tokens by top-k assignment" done on the host). The host
routes tokens (fp32, bit-matching the reference selection), assigns
tokens to cores greedily so per-(core, expert) counts are nearly equal,
derives per-expert slot capacities CAPS[e] from the realized maxima, and
ships each core its tokens PRE-SORTED into expert-major slot order
(d-major layout, bf16), so the device runs no gather/compaction at all.

On device: the two big GEMMs per expert start immediately (slot 0 data +
first weight quarter land ~10us in); the router reruns per-slot in bf16
(logitsT = rw.T @ xg, stationary rw) to compute the renormalized top-2
gate p = sigmoid(2*l_own - m1 - m2) — a smooth function of logits, so
bf16 is safe (the discrete top-2 selection lives on the host) — and the
gate is fused into the GEMM2 bias epilogue ((psum + b2) * gate on DVE).
Gated y-slots stream out; the host adds rows into the final output
(pure unshard work: the slot->token map is the host's own sharding).

Self-contained: hardcodes shapes B=4, S=2048, D=1024, F=4096, E=8, K=2.
"""
import sys

for p in ("/opt/trn_rl_repo",):
    if p not in sys.path:
        sys.path.insert(0, p)

import numpy as np
import ml_dtypes

import concourse.mybir as mybir
from concourse import bacc
from concourse.bass_utils import run_bass_kernel_spmd
from concourse.tile import TileContext

B, S, D, F, E = 4, 2048, 1024, 4096, 8
N = B * S            # 8192 tokens total
NC = 8               # cores
NT = N // NC         # 1024 tokens per core
KD = D // 128        # 8 contraction tiles over D
MF = F // 128        # 32 f tiles
NQ = 4               # weight streaming quarters per expert per GEMM
SUBF = F // NQ       # 1024 f columns per w1 quarter
SUBD = D // NQ       # 256 d columns per w2 quarter
F32 = mybir.dt.float32
BF16 = mybir.dt.bfloat16
NPBF16 = ml_dtypes.bfloat16

_GELU = mybir.ActivationFunctionType.Gelu


def build_nc(caps):
    """caps: per-expert slot capacities (each %16)."""
    caps = list(caps)
    assert len(caps) == E and all(c % 16 == 0 for c in caps)
    CAPX = max(caps)               # uniform per-expert stride
    NS = E * CAPX                  # unified slot space
    NCH = NS // 128                # 128-slot chunks (NS % 128 == 0)
    assert NS % 128 == 0

    nc = bacc.Bacc()
    xg_dr = nc.declare_dram_parameter("xg", [E, 128, KD * CAPX], BF16,
                                      isOutput=False)
    rw_dr = nc.declare_dram_parameter("rw", [128, KD * E], BF16, isOutput=False)
    rb_dr = nc.declare_dram_parameter("rb", [E, 1], F32, isOutput=False)
    w1_dr = nc.declare_dram_parameter("w1b", [E, NQ, 128, KD * SUBF], BF16,
                                      isOutput=False)
    # expert 0's first quarter in m-tile-major layout: each 256KB m-tile is
    # one contiguous DMA, so the very first GEMM matmuls start ~10us in
    w1m_dr = nc.declare_dram_parameter("w1e0m", [SUBF // 128, 128, KD * 128],
                                       BF16, isOutput=False)
    b1_dr = nc.declare_dram_parameter("b1r", [E, 128, MF], F32, isOutput=False)
    w2_dr = nc.declare_dram_parameter("w2b", [E, NQ, 128, MF * SUBD], BF16,
                                      isOutput=False)
    b2_dr = nc.declare_dram_parameter("b2r", [E, 128, KD], F32, isOutput=False)
    id_dr = nc.declare_dram_parameter("ident", [128, 128], F32, isOutput=False)
    om_dr = nc.declare_dram_parameter("ownmask", [128, NCH * E], F32,
                                      isOutput=False)
    out_dr = nc.declare_dram_parameter("out", [E, 128, KD * CAPX], F32,
                                       isOutput=True)

    with TileContext(nc) as tc:
        with tc.tile_pool(name="persist", bufs=1) as pp:
            # consts on the scalar queue: the sync queue leads with weights
            rw_sb = pp.tile([128, KD, E], BF16)
            nc.scalar.dma_start(out=rw_sb[:].rearrange("p k e -> p (k e)"),
                                in_=rw_dr[:])
            rb_sb = pp.tile([E, 1], F32)
            nc.scalar.dma_start(out=rb_sb[:], in_=rb_dr[:])
            ident = pp.tile([128, 128], F32)
            nc.scalar.dma_start(out=ident[:], in_=id_dr[:])
            ownmask = pp.tile([128, NCH, E], F32)
            nc.scalar.dma_start(out=ownmask[:].rearrange("p c e -> p (c e)"),
                                in_=om_dr[:])
            ones_row = pp.tile([1, 128], F32)
            nc.vector.memset(ones_row[:], 1.0)

            # pre-sorted tokens, d-major: xg[p, e, k, s] = x[tok(e,s), 128k+p]
            # on the gpsimd queue so the weight trigger stream (sync) and
            # activations (scalar) aren't behind it
            xg = pp.tile([128, E, KD, CAPX], BF16)
            for e in range(E):
                nc.gpsimd.dma_start(out=xg[:, e, :, :].rearrange("p k s -> p (k s)"),
                                    in_=xg_dr[e])

            probs_all = pp.tile([128, NS], F32)
            pflat = pp.tile([1, NS], F32)

            with (
                tc.tile_pool(name="rt_sb", bufs=2) as rsp,
                tc.tile_pool(name="rt1_sb", bufs=1) as rp1,
                tc.tile_pool(name="ps_rt", bufs=2, space="PSUM") as prt,
                tc.tile_pool(name="w1p", bufs=3) as wp1,
                tc.tile_pool(name="w1pm", bufs=3) as wp1m,
                tc.tile_pool(name="w2p", bufs=3) as wp2,
                tc.tile_pool(name="ht", bufs=1) as hp,
                tc.tile_pool(name="yt", bufs=2) as yp,
                tc.tile_pool(name="bias", bufs=2) as bp,
                tc.tile_pool(name="ps_g1", bufs=3, space="PSUM") as ps1,
                tc.tile_pool(name="ps_g2", bufs=3, space="PSUM") as ps2,
            ):
                lgs = rp1.tile([8, NS], F32)       # per-slot logitsT
                lg_all = rp1.tile([128, NCH, E], F32)

                def emit_router():
                    # logitsT = rw.T @ xg per expert block, bf16, rb added
                    for e in range(E):
                        psr = prt.tile([128, 512], F32, tag="rt", name=f"rt{e}")
                        for k in range(KD):
                            nc.tensor.matmul(psr[:8, :caps[e]], rw_sb[:, k, :],
                                             xg[:, e, k, :caps[e]],
                                             start=(k == 0), stop=(k == KD - 1))
                        nc.vector.tensor_scalar(
                            out=lgs[:, e * CAPX:e * CAPX + caps[e]],
                            in0=psr[:8, :caps[e]], scalar1=rb_sb[:, 0:1],
                            scalar2=None, op0=mybir.AluOpType.add)
                    for c in range(NCH):
                        tps = prt.tile([128, 512], F32, tag="rt", name=f"tp{c}")
                        nc.tensor.transpose(tps[:, :8], lgs[:, c * 128:(c + 1) * 128],
                                            ident[:8, :8])
                        nc.vector.tensor_copy(lg_all[:, c, :], tps[:, :8])

                def emit_probs():
                    # p = sigmoid(2*l_own - m1 - m2) per slot (smooth in logits)
                    m1 = rp1.tile([128, NCH], F32)
                    nc.vector.tensor_reduce(m1[:], lg_all[:],
                                            axis=mybir.AxisListType.X,
                                            op=mybir.AluOpType.max)
                    is1 = rp1.tile([128, NCH, E], F32)
                    nc.vector.tensor_tensor(
                        out=is1[:], in0=lg_all[:],
                        in1=m1[:].unsqueeze(2).broadcast_to([128, NCH, E]),
                        op=mybir.AluOpType.is_equal)
                    l2 = rp1.tile([128, NCH, E], F32)
                    nc.vector.scalar_tensor_tensor(out=l2[:], in0=is1[:],
                                                   scalar=-1e30, in1=lg_all[:],
                                                   op0=mybir.AluOpType.mult,
                                                   op1=mybir.AluOpType.add)
                    m2 = rp1.tile([128, NCH], F32)
                    nc.vector.tensor_reduce(m2[:], l2[:],
                                            axis=mybir.AxisListType.X,
                                            op=mybir.AluOpType.max)
                    lo = rp1.tile([128, NCH, E], F32)
                    nc.vector.tensor_tensor(out=lo[:], in0=lg_all[:], in1=ownmask[:],
                                            op=mybir.AluOpType.mult)
                    low = rp1.tile([128, NCH], F32)
                    nc.vector.tensor_reduce(low[:], lo[:],
                                            axis=mybir.AxisListType.X,
                                            op=mybir.AluOpType.add)
                    arg = rp1.tile([128, NCH], F32)
                    nc.vector.tensor_tensor(out=arg[:], in0=m1[:], in1=m2[:],
                                            op=mybir.AluOpType.add)
                    nc.vector.scalar_tensor_tensor(out=arg[:], in0=low[:],
                                                   scalar=2.0, in1=arg[:],
                                                   op0=mybir.AluOpType.mult,
                                                   op1=mybir.AluOpType.subtract)
                    pch = rp1.tile([128, NCH], F32)
                    nc.scalar.activation(pch[:], arg[:],
                                         mybir.ActivationFunctionType.Sigmoid,
                                         bias=0.0, scale=1.0)
                    # [128, NCH] -> [NCH, 128] -> flat [1, NS] -> bcast [128, NS]
                    tpp = prt.tile([128, 512], F32, tag="rt", name="tq")
                    nc.tensor.transpose(tpp[:NCH, :128], pch[:], ident[:])
                    pT = rp1.tile([NCH, 128], F32)
                    nc.vector.tensor_copy(pT[:], tpp[:NCH, :128])
                    for c in range(NCH):
                        nc.scalar.dma_start(out=pflat[0:1, c * 128:(c + 1) * 128],
                                            in_=pT[c:c + 1, :])
                    for e in range(E):
                        pb = prt.tile([128, 512], F32, tag="rt", name=f"pb{e}")
                        assert caps[e] <= 512
                        nc.tensor.matmul(pb[:, :caps[e]], ones_row[:],
                                         pflat[0:1, e * CAPX:e * CAPX + caps[e]],
                                         start=True, stop=True)
                        nc.vector.tensor_copy(
                            probs_all[:, e * CAPX:e * CAPX + caps[e]],
                            pb[:, :caps[e]])

                for e in range(E):
                    cap = caps[e]
                    b1_sb = bp.tile([128, MF], F32, tag="b1")
                    b2_sb = bp.tile([128, KD], F32, tag="b2")
                    if e == 0:
                        # first weight m-tile outranks the biases on the queue
                        w1m0 = wp1m.tile([128, KD, 128], BF16, tag="w1m")
                        nc.sync.dma_start(out=w1m0[:].rearrange("p k f -> p (k f)"),
                                          in_=w1m_dr[0])
                    nc.sync.dma_start(out=b1_sb[:], in_=b1_dr[e])
                    nc.sync.dma_start(out=b2_sb[:], in_=b2_dr[e])

                    # GEMM1 + bias + gelu -> hT [128, MF, cap] bf16
                    hT = hp.tile([128, MF, CAPX], BF16, tag="hT")
                    for q in range(NQ):
                        if not (e == 0 and q == 0):
                            w1q = wp1.tile([128, KD, SUBF], BF16, tag="w1q")
                            nc.sync.dma_start(out=w1q[:].rearrange("p k f -> p (k f)"),
                                              in_=w1_dr[e, q])
                        for mi in range(SUBF // 128):
                            m = q * (SUBF // 128) + mi
                            if e == 0 and q == 0:
                                if mi == 0:
                                    w1m = w1m0
                                else:
                                    w1m = wp1m.tile([128, KD, 128], BF16, tag="w1m")
                                    nc.sync.dma_start(
                                        out=w1m[:].rearrange("p k f -> p (k f)"),
                                        in_=w1m_dr[mi])
                                wsl = w1m
                                wof = 0
                            else:
                                wsl = w1q
                                wof = mi * 128
                            ps = ps1.tile([128, CAPX], F32, tag="g1",
                                          name=f"g1_{e}_{m}")
                            for k in range(KD):
                                nc.tensor.matmul(
                                    ps[:, :cap],
                                    wsl[:, k, wof:wof + 128],
                                    xg[:, e, k, :cap],
                                    start=(k == 0), stop=(k == KD - 1))
                            nc.scalar.activation(hT[:, m, :cap], ps[:, :cap],
                                                 _GELU, bias=b1_sb[:, m:m + 1],
                                                 scale=1.0)
                        if e == 0 and q == 2:
                            emit_router()
                        if e == 0 and q == 3:
                            emit_probs()

                    # GEMM2 with fused (psum + b2) * gate epilogue
                    yT = yp.tile([128, KD, CAPX], F32, tag="yT")
                    for dq in range(NQ):
                        w2q = wp2.tile([128, MF, SUBD], BF16, tag="w2q")
                        nc.sync.dma_start(out=w2q[:].rearrange("p k d -> p (k d)"),
                                          in_=w2_dr[e, dq])
                        for mi in range(SUBD // 128):
                            m = dq * (SUBD // 128) + mi
                            ps = ps2.tile([128, CAPX], F32, tag="g2",
                                          name=f"g2_{e}_{m}")
                            for k2 in range(MF):
                                nc.tensor.matmul(
                                    ps[:, :cap],
                                    w2q[:, k2, mi * 128:(mi + 1) * 128],
                                    hT[:, k2, :cap],
                                    start=(k2 == 0), stop=(k2 == MF - 1))
                            nc.vector.scalar_tensor_tensor(
                                out=yT[:, m, :cap], in0=ps[:, :cap],
                                scalar=b2_sb[:, m:m + 1],
                                in1=probs_all[:, e * CAPX:e * CAPX + cap],
                                op0=mybir.AluOpType.add,
                                op1=mybir.AluOpType.mult)
                        m0 = dq * (SUBD // 128)
                        if e == E - 1:
                            # finer final-expert writes shorten the tail drain
                            for m in range(m0, m0 + SUBD // 128):
                                nc.gpsimd.dma_start(
                                    out=out_dr[e][:, m * CAPX:(m + 1) * CAPX],
                                    in_=yT[:, m, :])
                        else:
                            nc.gpsimd.dma_start(
                                out=out_dr[e][:, m0 * CAPX:(m0 + SUBD // 128) * CAPX],
                                in_=yT[:, m0:m0 + SUBD // 128, :]
                                .rearrange("p a b -> p (a b)"))

    nc.finalize()
    return nc


def route_and_balance(x_flat, router_w, router_b):
    """Host fp32 routing (matches the reference selection) + greedy packing."""
    rwf = np.asarray(router_w, dtype=np.float32)
    logits = x_flat @ rwf + np.asarray(router_b, dtype=np.float32)
    top2 = np.argsort(-logits, axis=1)[:, :2]
    cnt = np.zeros((NC, E), np.int64)
    room = np.full(NC, NT, np.int64)
    perm = [[] for _ in range(NC)]
    for t in range(N):
        a, b = top2[t]
        best, bkey = None, None
        for c in range(NC):
            if room[c] == 0:
                continue
            key = (max(cnt[c, a], cnt[c, b]), cnt[c, a] + cnt[c, b], NT - room[c])
            if bkey is None or key < bkey:
                bkey, best = key, c
        perm[best].append(t)
        cnt[best, a] += 1
        cnt[best, b] += 1
        room[best] -= 1
    perm = np.array(perm, dtype=np.int64)
    caps = tuple(int(-(-(int(cnt[:, e].max())) // 16) * 16) for e in range(E))
    return perm, top2, caps


def make_in_maps(x, router_w, router_b, w1, b1, w2, b2):
    x_flat = np.ascontiguousarray(np.asarray(x, dtype=np.float32).reshape(N, D))
    perm, top2, caps = route_and_balance(x_flat, router_w, router_b)
    CAPX = max(caps)
    NS = E * CAPX
    NCH = NS // 128
    ident = np.eye(128, dtype=np.float32)
    b1r = np.ascontiguousarray(
        np.asarray(b1, dtype=np.float32).reshape(E, MF, 128).transpose(0, 2, 1))
    b2r = np.ascontiguousarray(
        np.asarray(b2, dtype=np.float32).reshape(E, KD, 128).transpose(0, 2, 1))
    w1b = np.ascontiguousarray(
        np.asarray(w1, dtype=np.float32).reshape(E, KD, 128, NQ, SUBF)
        .transpose(0, 3, 2, 1, 4).reshape(E, NQ, 128, KD * SUBF)
        .astype(NPBF16))
    w2b = np.ascontiguousarray(
        np.asarray(w2, dtype=np.float32).reshape(E, MF, 128, NQ, SUBD)
        .transpose(0, 3, 2, 1, 4).reshape(E, NQ, 128, MF * SUBD)
        .astype(NPBF16))
    # w1e0m[mi, p, (k, c)] = w1[0, 128k + p, 128mi + c]  (m-tile-major q0)
    w1e0m = np.ascontiguousarray(
        np.asarray(w1[0], dtype=np.float32).reshape(KD, 128, MF, 128)
        .transpose(2, 1, 0, 3)[:SUBF // 128].reshape(SUBF // 128, 128, KD * 128)
        .astype(NPBF16))
    rw_re = np.ascontiguousarray(
        np.asarray(router_w, dtype=np.float32).reshape(KD, 128, E)
        .transpose(1, 0, 2).reshape(128, KD * E)).astype(NPBF16)
    rb_re = np.ascontiguousarray(
        np.asarray(router_b, dtype=np.float32).reshape(E, 1))
    common = dict(rw=rw_re, rb=rb_re, w1b=w1b, w1e0m=w1e0m, b1r=b1r, w2b=w2b,
                  b2r=b2r, ident=ident)
    in_maps = []
    slot_tok = []      # per core: local token id per slot (-1 = pad)
    for c in range(NC):
        m = dict(common)
        xs = x_flat[perm[c]].astype(NPBF16)
        t2c = top2[perm[c]]    # [NT, 2] expert pairs of this core's tokens
        st = np.full((E, CAPX), -1, np.int64)
        fill = np.zeros(E, np.int64)
        for lid in range(NT):
            for e in t2c[lid]:
                st[e, fill[e]] = lid
                fill[e] += 1
        slot_tok.append(st)
        stc = np.where(st < 0, 0, st)    # pad slots read token 0 (ignored)
        xsl = xs[stc.reshape(-1)]        # [NS, D] bf16
        # xg[e, p, (k, s)] = xsl[e*CAPX + s, 128k + p]
        m["xg"] = np.ascontiguousarray(
            xsl.reshape(E, CAPX, KD, 128).transpose(0, 3, 2, 1)
            .reshape(E, 128, KD * CAPX))
        # ownmask[p, c, e] = 1 iff slot c*128+p belongs to expert e (not pad)
        om = np.zeros((128, NCH, E), np.float32)
        sl = np.arange(NS)
        eo = sl // CAPX
        valid = (st.reshape(-1) >= 0)
        om[sl % 128, sl // 128, eo] = valid.astype(np.float32)
        m["ownmask"] = np.ascontiguousarray(om.reshape(128, NCH * E))
        in_maps.append(m)
    return in_maps, perm, caps, slot_tok


def assemble(results, perm, caps, slot_tok):
    CAPX = max(caps)
    out = np.zeros((N, D), np.float32)
    for c in range(NC):
        y = results[c]["out"].reshape(E, 128, KD, CAPX)
        for e in range(E):
            k_e = int((slot_tok[c][e] >= 0).sum())
            rows = np.ascontiguousarray(
                y[e, :, :, :k_e].transpose(2, 1, 0).reshape(k_e, D))
            np.add.at(out, perm[c][slot_tok[c][e][:k_e]], rows)
    return out


_nc_cache = {}


def get_nc(caps):
    if caps not in _nc_cache:
        _nc_cache[caps] = build_nc(caps)
    return _nc_cache[caps]


def kernel(x, router_w, router_b, w1, b1, w2, b2, **extra):
    in_maps, perm, caps, slot_tok = make_in_maps(x, router_w, router_b,
                                                 w1, b1, w2, b2)
    nc = get_nc(caps)
    res = run_bass_kernel_spmd(nc, in_maps, list(range(NC)))
    out = assemble(res.results, perm, caps, slot_tok)
    return out.reshape(B, S, D)
